# revision 1
# baseline (speedup 1.0000x reference)
"""Trainium2 Bass kernel for nn_DigitConvolutionalModel (dense_cnn).

Math: the 3x3 valid conv is linear in x, so it folds into fc1:
    conv(x) @ fc1_w.T == x @ (C @ fc1_w.T)  with C [784, 676] the conv matrix.
The whole model is then a 3-layer MLP:
    out = relu(relu(x @ W1 + b1) @ W2 + b2) @ W3 + b3
with W1 = C @ fc1_w.T [784,512], W2 = fc2_w.T [512,512], W3 = out_w.T [512,10].

Sharding: pure data parallelism; batch 32768 -> 8 cores x 4096 rows.

On-chip formulation is fully transposed (features on SBUF partitions, batch on
the free dim), so the big activation matrices never need an on-chip transpose:
the host passes x^T slices, and each layer computes
    h^T = act(W_l^T-as-lhsT . T @ h_{l-1}^T + b)     (PE: out = lhsT.T @ rhs)
with lhsT = W_l tiles ([K<=128 part, M<=128 free]) and rhs = previous h^T.
Biases vary along partitions -> per-partition scalar bias on the ACT engine.

Per core: batch 4096 processed in 8 chunks of N=512 (moving free dim / one
PSUM bank). Layer1 K = 784 = 7 k-tiles of 112 partitions; layers 2/3 K = 512
= 4 k-tiles of 128. Matmul dtype is bf16 (fp32 PSUM accumulation), which
measured 2.5e-3 L2 rel err end-to-end vs the fp32 reference.
"""

import numpy as np
import ml_dtypes

NCORES = 8
B = 32768
BC = B // NCORES  # rows per core
CH = 512          # batch chunk = matmul moving free dim = one fp32 PSUM bank
NCH = BC // CH
KP1, KT1 = 112, 7  # layer-1 contraction tiling: 784 = 7 * 112
MT1 = 4            # 512 out feats = 4 m-tiles of 128
KT2, MT2 = 4, 4    # layer-2: K=512, M=512
KT3, MO = 4, 10    # layer-3: K=512, M=10

# "bf16" or "fp32r" (fp32 storage, reduced-precision full-rate matmul)
MM_DTYPE = "bf16"

_cache = {}


def _np_dtype():
    return ml_dtypes.bfloat16 if MM_DTYPE == "bf16" else np.float32


def _build():
    """Trace + compile the Bass program once per process."""
    if "nc" in _cache:
        return _cache["nc"]

    from contextlib import ExitStack

    import concourse.bass as bass
    import concourse.tile as tile
    from concourse import bacc, mybir
    from concourse.bass import ts

    DT = mybir.dt.bfloat16 if MM_DTYPE == "bf16" else mybir.dt.float32r
    F32 = mybir.dt.float32
    Relu = mybir.ActivationFunctionType.Relu
    Ident = mybir.ActivationFunctionType.Identity
    # 2-byte dtype keeps all 8 x^T chunks resident; 4-byte streams through 5
    xt_bufs = NCH if mybir.dt.size(DT) == 2 else 5

    nc = bacc.Bacc(
        "TRN2",
        target_bir_lowering=False,
        debug=False,
        enable_asserts=False,
        num_devices=NCORES,
    )

    xt_d = nc.dram_tensor("xt", [KT1, KP1, BC], DT, kind="ExternalInput")
    w1_d = nc.dram_tensor("w1", [KT1, KP1, 512], DT, kind="ExternalInput")
    w2_d = nc.dram_tensor("w2", [KT2, 128, 512], DT, kind="ExternalInput")
    w3_d = nc.dram_tensor("w3", [128, KT3 * MO], DT, kind="ExternalInput")
    b1_d = nc.dram_tensor("b1", [128, MT1], F32, kind="ExternalInput")
    b2_d = nc.dram_tensor("b2", [128, MT2], F32, kind="ExternalInput")
    b3_d = nc.dram_tensor("b3", [MO, 1], F32, kind="ExternalInput")
    out_d = nc.dram_tensor("out", [MO, BC], F32, kind="ExternalOutput")

    with tile.TileContext(nc) as tc, ExitStack() as ctx:
        consts = ctx.enter_context(tc.tile_pool(name="consts", bufs=1))
        xt_pool = ctx.enter_context(tc.tile_pool(name="xt", bufs=xt_bufs))
        h1_pool = ctx.enter_context(tc.tile_pool(name="h1", bufs=3))
        h2_pool = ctx.enter_context(tc.tile_pool(name="h2", bufs=3))
        ps1 = ctx.enter_context(tc.tile_pool(name="ps1", bufs=4, space="PSUM"))
        ps2 = ctx.enter_context(tc.tile_pool(name="ps2", bufs=2, space="PSUM"))
        ps3 = ctx.enter_context(tc.tile_pool(name="ps3", bufs=2, space="PSUM"))

        w1_sb = consts.tile([KP1, KT1, 512], DT, name="w1_sb")
        nc.sync.dma_start(w1_sb[:], w1_d.rearrange("t p m -> p t m"))

        xtc = []
        t0 = xt_pool.tile([KP1, KT1, CH], DT, name="xtc0", tag="xtc")
        nc.sync.dma_start(
            t0[:], xt_d.rearrange("t p n -> p t n")[:, :, ts(0, CH)]
        )
        xtc.append(t0)

        b1_sb = consts.tile([128, MT1], F32, name="b1_sb")
        nc.sync.dma_start(b1_sb[:], b1_d[:])
        w2_sb = consts.tile([128, KT2, 512], DT, name="w2_sb")
        nc.sync.dma_start(w2_sb[:], w2_d.rearrange("t p m -> p t m"))
        b2_sb = consts.tile([128, MT2], F32, name="b2_sb")
        nc.sync.dma_start(b2_sb[:], b2_d[:])
        w3_sb = consts.tile([128, KT3 * MO], DT, name="w3_sb")
        nc.sync.dma_start(w3_sb[:], w3_d[:])
        b3_sb = consts.tile([MO, 1], F32, name="b3_sb")
        nc.sync.dma_start(b3_sb[:], b3_d[:])
        outb = consts.tile([MO, BC], F32, name="outb")

        for n in range(1, NCH):
            t = xt_pool.tile([KP1, KT1, CH], DT, name=f"xtc{n}", tag="xtc")
            nc.sync.dma_start(
                t[:], xt_d.rearrange("t p n -> p t n")[:, :, ts(n, CH)]
            )
            xtc.append(t)

        def layer1(n):
            h1t = h1_pool.tile([128, MT1, CH], DT, name=f"h1_{n}", tag="h1")
            for mi in range(MT1):
                ps = ps1.tile([128, CH], F32, name=f"ps1_{n}_{mi}", tag="ps1")
                for ki in range(KT1):
                    nc.tensor.matmul(
                        ps[:],
                        w1_sb[:, ki, ts(mi, 128)],
                        xtc[n][:, ki, :],
                        start=(ki == 0),
                        stop=(ki == KT1 - 1),
                    )
                nc.scalar.activation(
                    h1t[:, mi, :], ps[:], Relu, bias=b1_sb[:, mi : mi + 1]
                )
            return h1t

        def layer23(n, h1t):
            h2t = h2_pool.tile([128, MT2, CH], DT, name=f"h2_{n}", tag="h2")
            for mi in range(MT2):
                ps = ps2.tile([128, CH], F32, name=f"ps2_{n}_{mi}", tag="ps2")
                for ki in range(KT2):
                    nc.tensor.matmul(
                        ps[:],
                        w2_sb[:, ki, ts(mi, 128)],
                        h1t[:, ki, :],
                        start=(ki == 0),
                        stop=(ki == KT2 - 1),
                    )
                nc.scalar.activation(
                    h2t[:, mi, :], ps[:], Relu, bias=b2_sb[:, mi : mi + 1]
                )
            ps = ps3.tile([MO, CH], F32, name=f"ps3_{n}", tag="ps3")
            for ki in range(KT3):
                nc.tensor.matmul(
                    ps[:],
                    w3_sb[:, ts(ki, MO)],
                    h2t[:, ki, :],
                    start=(ki == 0),
                    stop=(ki == KT3 - 1),
                )
            nc.scalar.activation(
                outb[:, ts(n, CH)], ps[:], Ident, bias=b3_sb[:, 0:1]
            )

        # Software pipeline: PE never waits on ACT-produced h1 of the same
        # chunk — layer2/3 of chunk n-1 run while layer1 of chunk n fills.
        prev = None
        for n in range(NCH):
            h1t = layer1(n)
            if prev is not None:
                layer23(n - 1, prev)
            prev = h1t
        layer23(NCH - 1, prev)

        nc.sync.dma_start(out_d[:], outb[:])

    nc.compile()
    _cache["nc"] = nc
    return nc


def _prep_inputs(x, conv_w, fc1_w, fc1_b, fc2_w, fc2_b, out_w, out_b):
    dt = _np_dtype()
    f32 = np.float32

    # Conv as a [784, 676] matrix (exact in fp64), folded into fc1.
    C = np.zeros((784, 676), dtype=np.float64)
    oy, ox = np.meshgrid(np.arange(26), np.arange(26), indexing="ij")
    cols = (oy * 26 + ox).ravel()
    for ky in range(3):
        for kx in range(3):
            rows = ((oy + ky) * 28 + (ox + kx)).ravel()
            np.add.at(C, (rows, cols), float(conv_w[ky, kx]))
    W1 = (C @ fc1_w.T.astype(np.float64)).astype(f32)  # [784, 512]

    w1 = np.ascontiguousarray(W1).reshape(KT1, KP1, 512).astype(dt)
    w2 = np.ascontiguousarray(fc2_w.T).reshape(KT2, 128, 512).astype(dt)
    # [512,10] -> [4,128,10] -> [128, 4*10] so each partition is one 80B run
    w3 = np.ascontiguousarray(
        np.ascontiguousarray(out_w.T).reshape(KT3, 128, MO).transpose(1, 0, 2)
    ).reshape(128, KT3 * MO).astype(dt)
    b1 = np.ascontiguousarray(fc1_b.reshape(MT1, 128).T).astype(f32)
    b2 = np.ascontiguousarray(fc2_b.reshape(MT2, 128).T).astype(f32)
    b3 = np.ascontiguousarray(out_b.reshape(MO, 1)).astype(f32)

    in_maps = []
    for c in range(NCORES):
        xc = x[c * BC : (c + 1) * BC].T.astype(dt, order="C")  # [784, BC]
        in_maps.append(
            {
                "xt": xc.reshape(KT1, KP1, BC),
                "w1": w1,
                "w2": w2,
                "w3": w3,
                "b1": b1,
                "b2": b2,
                "b3": b3,
            }
        )
    return in_maps


def kernel(x, conv_w, fc1_w, fc1_b, fc2_w, fc2_b, out_w, out_b, _results=None):
    from concourse.bass_utils import run_bass_kernel_spmd

    nc = _build()
    in_maps = _prep_inputs(x, conv_w, fc1_w, fc1_b, fc2_w, fc2_b, out_w, out_b)
    res = run_bass_kernel_spmd(nc, in_maps, core_ids=list(range(NCORES)))
    if _results is not None:
        _results.append(res)
    out = np.empty((B, 10), dtype=np.float32)
    for c in range(NCORES):
        out[c * BC : (c + 1) * BC, :] = res.results[c]["out"].T
    return out
